# revision 1
# baseline (speedup 1.0000x reference)
"""Trainium2 Bass kernel for pre-LN multi-head self-attention (nn_Attn).

Shapes (hardcoded): x [4, 2048, 1024], 16 heads x 64 head_dim, fp32.
Sharding: tensor-parallel over heads -- core c owns heads {2c, 2c+1};
each core computes LN + its QKV slice + attention + a partial out-
projection; host sums the 8 partials and adds b_out.

All matmuls run in float32r (tf32-grade, ~1.5e-4 rel err, 4x faster
than fp32 on the PE).  Dataflow is transposed: zT [D, tok] feeds
Wqkv^T @ zT -> Q^T/K^T/V^T; scores S^T[k,q] = K Q^T; softmax runs
exp on ACT straight out of PSUM; PV uses V augmented with a ones
column so softmax denominators fall out of the same matmul; attnout
is normalized during the PSUM->SBUF copy via a broadcast reciprocal
row, so the final projection is one K=128 matmul per tile.
"""

import numpy as np

B = 4
S = 2048
DIM = 1024
HEADS = 16
HD = 64
N_CORES = 8
TOK = B * S  # 8192
EPS = 1e-5
SCALE = HD ** -0.5

_CACHE = {}


def _build_program():
    import concourse.bass as bass
    import concourse.mybir as mybir
    import concourse.tile as tile
    from concourse import bacc

    f32 = mybir.dt.float32
    f32r = mybir.dt.float32r
    AF = mybir.ActivationFunctionType
    OP = mybir.AluOpType

    nc = bacc.Bacc("TRN2", target_bir_lowering=False, debug=False,
                   num_devices=N_CORES)

    x = nc.dram_tensor("x", [TOK, DIM], f32, kind="ExternalInput")
    wq = nc.dram_tensor("wq", [DIM, 128], f32, kind="ExternalInput")
    wk = nc.dram_tensor("wk", [DIM, 128], f32, kind="ExternalInput")
    wv = nc.dram_tensor("wv", [DIM, 128], f32, kind="ExternalInput")
    bqkv = nc.dram_tensor("bqkv", [3, 128], f32, kind="ExternalInput")
    wo = nc.dram_tensor("wo", [128, DIM], f32, kind="ExternalInput")
    ident = nc.dram_tensor("ident", [128, 128], f32, kind="ExternalInput")
    y = nc.dram_tensor("y", [TOK, DIM], f32, kind="ExternalOutput")
    dscr = nc.dram_tensor("dscr", [B, 2, 4, 512], f32)

    from contextlib import ExitStack
    with tile.TileContext(nc) as tc:
      with ExitStack() as ctx:
        P = lambda **kw: ctx.enter_context(tc.tile_pool(**kw))
        singles = P(name="singles", bufs=1)
        xt_pool = P(name="xt", bufs=2)
        st_pool = P(name="stats", bufs=6)
        z_pool = P(name="z", bufs=2)
        zT_pool = P(name="zT", bufs=1)
        qkvT_pool = P(name="qkvT", bufs=2)
        vT_pool = P(name="vT", bufs=1)
        vaug_pool = P(name="vaug", bufs=1)
        pt_pool = P(name="pt", bufs=2)
        ao_pool = P(name="ao", bufs=1)
        den_pool = P(name="den", bufs=2)
        y_pool = P(name="ysb", bufs=3)
        s_ps = P(name="s_ps", bufs=2, space="PSUM")
        mm_ps = P(name="mm_ps", bufs=2, space="PSUM")
        tr_ps = P(name="tr_ps", bufs=1, space="PSUM")
        pv_ps = P(name="pv_ps", bufs=1, space="PSUM")
        if True:
            # --- weights / constants resident in SBUF (fp32r via SWDGE cast)
            w_sb = []
            for m, w in enumerate((wq, wk, wv)):
                t = singles.tile([128, 8, 128], f32r, tag=f"w{m}")
                nc.gpsimd.dma_start(
                    out=t, in_=w.rearrange("(dc p) m -> p dc m", p=128))
                w_sb.append(t)
            bias_sb = singles.tile([3, 128], f32, tag="bias")
            nc.gpsimd.dma_start(out=bias_sb, in_=bqkv[:, :])
            # per-partition bias columns for the QKV^T copies: need [128, 1]
            # with partition = qkv-dim; bqkv rows are [3,128] (m, dim) so a
            # transposed view is required -> load as [128, 3] instead.
            biasT_sb = singles.tile([128, 3], f32, tag="biasT")
            nc.gpsimd.dma_start(
                out=biasT_sb, in_=bqkv.rearrange("m p -> p m"))
            wo_sb = singles.tile([128, 2, 512], f32r, tag="wo")
            nc.gpsimd.dma_start(
                out=wo_sb, in_=wo.rearrange("p (n c) -> p n c", c=512))
            id_sb = singles.tile([128, 128], f32r, tag="ident")
            nc.gpsimd.dma_start(out=id_sb, in_=ident[:, :])
            eps_sb = singles.tile([128, 1], f32, tag="eps")
            nc.vector.memset(eps_sb, EPS)

            for b in range(B):
                # =========== phase A: LN + transpose + QKV ===========
                zT = zT_pool.tile([128, 8, S], f32r, tag="zT")
                mvs = st_pool.tile([128, 16, 2], f32, tag="mvs")
                for tt in range(S // 128):
                    tok0 = b * S + tt * 128
                    xt = xt_pool.tile([128, DIM], f32, tag="x")
                    nc.sync.dma_start(out=xt, in_=x[tok0:tok0 + 128, :])
                    stats = st_pool.tile([128, 2, 6], f32, tag="bn")
                    for g in range(2):
                        nc.vector.bn_stats(out=stats[:, g, :],
                                           in_=xt[:, g * 512:(g + 1) * 512])
                    nc.vector.bn_aggr(out=mvs[:, tt, :], in_=stats)
                # one Ln + one Exp per batch keeps the ACT table set stable
                lnv = st_pool.tile([128, 16], f32, tag="lnv")
                nc.scalar.activation(out=lnv, in_=mvs[:, :, 1],
                                     func=AF.Ln, bias=eps_sb, scale=1.0)
                rstd = st_pool.tile([128, 16], f32, tag="rstd")
                nc.scalar.activation(out=rstd, in_=lnv, func=AF.Exp,
                                     scale=-0.5)
                for tt in range(S // 128):
                    tok0 = b * S + tt * 128
                    xt = xt_pool.tile([128, DIM], f32, tag="x")
                    nc.sync.dma_start(out=xt, in_=x[tok0:tok0 + 128, :])
                    zt = z_pool.tile([128, DIM], f32r, tag="z")
                    nc.vector.tensor_scalar(
                        out=zt, in0=xt, scalar1=mvs[:, tt, 0:1],
                        scalar2=rstd[:, tt:tt + 1],
                        op0=OP.subtract, op1=OP.mult)
                    # 8 PE transposes -> zT[:, dc, tt*128:+128]
                    for half in range(2):
                        tp = tr_ps.tile([128, 4, 128], f32r, tag="tr")
                        for j in range(4):
                            dc = half * 4 + j
                            nc.tensor.matmul(
                                out=tp[:, j, :],
                                lhsT=zt[:, dc * 128:(dc + 1) * 128],
                                rhs=id_sb, is_transpose=True,
                                start=(j == 0), stop=(j == 3),
                                skip_group_check=True)
                        nc.vector.tensor_copy(
                            zT[:, half * 4:(half + 1) * 4,
                               tt * 128:(tt + 1) * 128], tp)

                qT = qkvT_pool.tile([128, S], f32r, tag="qT")
                kT = qkvT_pool.tile([128, S], f32r, tag="kT")
                vT = vT_pool.tile([128, S], f32r, tag="vT")
                for m, dst in enumerate((qT, kT, vT)):
                    for ncol in range(S // 512):
                        ps = mm_ps.tile([128, 512], f32, tag="mm")
                        for dc in range(8):
                            nc.tensor.matmul(
                                ps, lhsT=w_sb[m][:, dc, :],
                                rhs=zT[:, dc, ncol * 512:(ncol + 1) * 512],
                                start=(dc == 0), stop=(dc == 7))
                        nc.vector.tensor_scalar(
                            out=dst[:, ncol * 512:(ncol + 1) * 512],
                            in0=ps, scalar1=biasT_sb[:, m:m + 1],
                            scalar2=None, op0=OP.add)

                # V natural (+ ones col) per head: vaug [128, 16, 65]
                vaug = []
                for h in range(2):
                    va = vaug_pool.tile([128, 16, 66], f32r, tag=f"va{h}")
                    nc.vector.memset(va.bitcast(f32), 1.0)
                    for q8 in range(2):
                        tp = tr_ps.tile([128, 8, 64], f32r, tag="tr")
                        for j in range(8):
                            kt_i = q8 * 8 + j
                            nc.tensor.matmul(
                                out=tp[:, j, :],
                                lhsT=vT[h * 64:(h + 1) * 64,
                                        kt_i * 128:(kt_i + 1) * 128],
                                rhs=id_sb[h * 64:h * 64 + 64,
                                          h * 64:h * 64 + 64],
                                is_transpose=True,
                                start=(j == 0), stop=(j == 7),
                                skip_group_check=True)
                        nc.vector.tensor_copy(
                            va[:, q8 * 8:(q8 + 1) * 8, 0:64], tp)
                    vaug.append(va)

                # =========== attention ===========
                ao = ao_pool.tile([128, S], f32r, tag="ao")
                for h in range(2):
                    hs = slice(h * 64, h * 64 + 64)
                    tpos = (h * 64, 0)
                    for qc in range(4):
                        qs = slice(qc * 512, qc * 512 + 512)
                        pv = pv_ps.tile([65, 512], f32, tag="pv")
                        for ktg in range(8):
                            sp = s_ps.tile([128, 2, 512], f32, tag="s")
                            for kt in range(2):
                                kt_i = ktg * 2 + kt
                                nc.tensor.matmul(
                                    sp[:, kt, :],
                                    lhsT=kT[hs, kt_i * 128:(kt_i + 1) * 128],
                                    rhs=qT[hs, qs],
                                    start=True, stop=True,
                                    tile_position=tpos)
                            pt = pt_pool.tile([128, 2, 512], f32r, tag="pt")
                            nc.scalar.activation(out=pt, in_=sp, func=AF.Exp)
                            for kt in range(2):
                                kt_i = ktg * 2 + kt
                                nc.tensor.matmul(
                                    pv, lhsT=vaug[h][:, kt_i, 0:65],
                                    rhs=pt[:, kt, :],
                                    start=(kt_i == 0), stop=(kt_i == 15))
                        # denominators -> reciprocal -> broadcast
                        dsb = den_pool.tile([1, 512], f32, tag="dsb")
                        nc.vector.tensor_copy(dsb, pv[64:65, :])
                        rec = den_pool.tile([1, 512], f32, tag="rec")
                        nc.vector.reciprocal_approx_fast(out=rec, in_=dsb)
                        nc.sync.dma_start(out=dscr[b, h, qc, :], in_=rec)
                        bc = den_pool.tile([64, 512], f32, tag="bc")
                        base = dscr[b, h, qc, :]
                        nc.gpsimd.dma_start(
                            out=bc,
                            in_=bass.AP(tensor=base.tensor, offset=base.offset,
                                        ap=[[0, 64]] + list(base.ap)))
                        nc.vector.tensor_tensor(
                            out=ao[hs, qs], in0=pv[0:64, :], in1=bc,
                            op=OP.mult)

                # =========== out-projection (partial; host adds b_out) ====
                for tt in range(S // 128):
                    tok0 = b * S + tt * 128
                    for ncol in range(2):
                        ps = mm_ps.tile([128, 512], f32, tag="mm")
                        nc.tensor.matmul(
                            ps, lhsT=ao[:, tt * 128:(tt + 1) * 128],
                            rhs=wo_sb[:, ncol, :], start=True, stop=True)
                        ys = y_pool.tile([128, 512], f32, tag="y")
                        nc.vector.tensor_copy(ys, ps)
                        nc.sync.dma_start(
                            out=y[tok0:tok0 + 128,
                                  ncol * 512:(ncol + 1) * 512],
                            in_=ys)

    nc.compile()
    return nc


def _get_program():
    if "nc" not in _CACHE:
        _CACHE["nc"] = _build_program()
    return _CACHE["nc"]


def kernel(x, ln_g, ln_b, w_qkv, b_qkv, w_out, b_out, _trace=False):
    from concourse.bass_utils import run_bass_kernel_spmd

    nc = _get_program()

    x = np.asarray(x, dtype=np.float32)
    ln_g = np.asarray(ln_g, dtype=np.float32)
    ln_b = np.asarray(ln_b, dtype=np.float32)
    w_qkv = np.asarray(w_qkv, dtype=np.float32)
    b_qkv = np.asarray(b_qkv, dtype=np.float32)
    w_out = np.asarray(w_out, dtype=np.float32)
    b_out = np.asarray(b_out, dtype=np.float32)

    b, s, d = x.shape
    x_flat = np.ascontiguousarray(x.reshape(TOK, DIM))

    # Fold LN affine into the QKV projection:
    #   xn = z * g + beta with z = (x - mu) * rstd
    #   xn @ W + b = z @ (diag(g) W) + (beta @ W + b)
    w_eff = w_qkv * ln_g[:, None]
    b_eff = b_qkv + ln_b @ w_qkv
    ident = np.eye(128, dtype=np.float32)

    in_maps = []
    for c in range(N_CORES):
        lo = c * 128
        sl = slice(lo, lo + 128)
        wq_c = np.ascontiguousarray(w_eff[:, sl] * SCALE)
        wk_c = np.ascontiguousarray(w_eff[:, 1024 + lo:1024 + lo + 128])
        wv_c = np.ascontiguousarray(w_eff[:, 2048 + lo:2048 + lo + 128])
        bqkv_c = np.stack([b_eff[sl] * SCALE,
                           b_eff[1024 + lo:1024 + lo + 128],
                           b_eff[2048 + lo:2048 + lo + 128]])
        wo_c = np.ascontiguousarray(w_out[sl, :])
        in_maps.append({
            "x": x_flat, "wq": wq_c, "wk": wk_c, "wv": wv_c,
            "bqkv": np.ascontiguousarray(bqkv_c), "wo": wo_c,
            "ident": ident,
        })

    res = run_bass_kernel_spmd(nc, in_maps, core_ids=list(range(N_CORES)),
                               trace=_trace)
    y = sum(r["y"].astype(np.float64) for r in res.results)
    y = (y + b_out.astype(np.float64)).astype(np.float32)
    if _trace:
        _CACHE["last_exec_time_ns"] = res.exec_time_ns
        _CACHE["last_results"] = res
    return y.reshape(b, s, d)

